# revision 1
# baseline (speedup 1.0000x reference)
"""FWHT (N=16384, orthonormal) over a (32, 64, 16384) f32 batch on 8 TRN2 cores.

Decomposition: H_16384 = H_128 (x) H_128.  Each length-16384 row reshaped to
X[i, j] (128x128) transforms as Y = H X H / 128.  On the PE (out = lhsT.T @ rhs):
  mm1: lhsT = X  (K=i), rhs = H      -> out1[j, a] = sum_i X[i,j] H[i,a]
  mm2: lhsT = out1 (K=j), rhs = H/128 -> out2[a, b] = Y[a, b]
No transposes needed anywhere; out2 lands in the natural row-major layout.

Sharding: pure data-parallel over the 2048 leading rows -> 256 rows/core.
"""

import numpy as np

import concourse.bass as bass
import concourse.bacc as bacc
import concourse.tile as tile
import concourse.mybir as mybir
from concourse.bass_utils import run_bass_kernel_spmd

N_CORES = 8
R = 256          # rows per core (2048 / 8)
BLK = 32         # rows per DMA block (32 * 64KB = 2 MiB per transfer)
GRP = 4          # rows per PSUM group (4 * 128 f32 = one 2KB PSUM bank)
NBLK = R // BLK
NGRP = BLK // GRP

_cache = {}
LAST_RESULTS = None


def _hadamard128() -> np.ndarray:
    idx = np.arange(128, dtype=np.uint32)
    bits = idx[:, None] & idx[None, :]
    pop = np.zeros_like(bits)
    for s in range(7):
        pop += (bits >> s) & 1
    return np.where(pop % 2 == 0, np.float32(1.0), np.float32(-1.0)).astype(np.float32)


def _build():
    nc = bacc.Bacc(
        "TRN2",
        target_bir_lowering=False,
        debug=False,
        num_devices=N_CORES,
    )
    f32 = mybir.dt.float32
    x_d = nc.dram_tensor("x", [R, 128, 128], f32, kind="ExternalInput").ap()
    h_d = nc.dram_tensor("h", [128, 128], f32, kind="ExternalInput").ap()
    hs_d = nc.dram_tensor("hs", [128, 128], f32, kind="ExternalInput").ap()
    y_d = nc.dram_tensor("y", [R, 128, 128], f32, kind="ExternalOutput").ap()

    with tile.TileContext(nc) as tc:
        with (
            tc.tile_pool(name="hconst", bufs=1) as hpool,
            tc.tile_pool(name="xin", bufs=3) as xpool,
            tc.tile_pool(name="yout", bufs=2) as ypool,
            tc.tile_pool(name="mid", bufs=4) as mpool,
            tc.tile_pool(name="ps1", bufs=3, space=bass.MemorySpace.PSUM) as ps1pool,
            tc.tile_pool(name="ps2", bufs=3, space=bass.MemorySpace.PSUM) as ps2pool,
        ):
            ht = hpool.tile([128, 128], f32)
            hst = hpool.tile([128, 128], f32)
            nc.sync.dma_start(ht[:], h_d[:])
            nc.sync.dma_start(hst[:], hs_d[:])

            for b in range(NBLK):
                xt = xpool.tile([128, BLK, 128], f32)
                nc.sync.dma_start(
                    xt[:], x_d[b * BLK : (b + 1) * BLK].rearrange("r i j -> i r j")
                )
                yt = ypool.tile([128, BLK, 128], f32)
                for g in range(NGRP):
                    ps1 = ps1pool.tile([128, GRP, 128], f32)
                    for k in range(GRP):
                        nc.tensor.matmul(
                            ps1[:, k, :],
                            xt[:, g * GRP + k, :],
                            ht[:],
                            start=True,
                            stop=True,
                        )
                    sb1 = mpool.tile([128, GRP, 128], f32)
                    nc.vector.tensor_copy(sb1[:], ps1[:])
                    ps2 = ps2pool.tile([128, GRP, 128], f32)
                    for k in range(GRP):
                        nc.tensor.matmul(
                            ps2[:, k, :],
                            sb1[:, k, :],
                            hst[:],
                            start=True,
                            stop=True,
                        )
                    nc.vector.tensor_copy(yt[:, g * GRP : (g + 1) * GRP, :], ps2[:])
                nc.scalar.dma_start(
                    y_d[b * BLK : (b + 1) * BLK].rearrange("r a b -> a r b"), yt[:]
                )

    nc.compile()
    return nc


def kernel(**inputs) -> np.ndarray:
    global LAST_RESULTS
    x = np.ascontiguousarray(np.asarray(inputs["x"], dtype=np.float32))
    B, C, N = x.shape
    assert (B, C, N) == (32, 64, 16384)

    if "nc" not in _cache:
        _cache["nc"] = _build()
    nc = _cache["nc"]

    H = _hadamard128()
    HS = (H / np.float32(128.0)).astype(np.float32)

    shards = x.reshape(N_CORES, R, 128, 128)
    in_maps = [
        {"x": np.ascontiguousarray(shards[c]), "h": H, "hs": HS}
        for c in range(N_CORES)
    ]
    res = run_bass_kernel_spmd(nc, in_maps, core_ids=list(range(N_CORES)))
    LAST_RESULTS = res
    y = np.concatenate([res.results[c]["y"].reshape(1, R, 16384) for c in range(N_CORES)])
    return y.reshape(B, C, N)


# revision 20
# speedup vs baseline: 1.1076x; 1.1076x over previous
"""FWHT (N=16384, orthonormal) over a (32, 64, 16384) f32 batch on 8 TRN2 cores.

Decomposition: H_16384 = H_128 (x) H_128.  Each length-16384 row reshaped to
X[i, j] (128x128) transforms as Y = H X H / 128.  On the PE (out = lhsT.T @ rhs):
  mm1: lhsT = X  (K=i), rhs = H      -> out1[j, a] = sum_i X[i,j] H[i,a]
  mm2: lhsT = out1 (K=j), rhs = H/128 -> out2[a, b] = Y[a, b]
No transposes needed anywhere; out2 lands in the natural row-major layout.

Sharding: pure data-parallel over the 2048 leading rows -> 256 rows/core.
"""

import numpy as np

import concourse.bass as bass
import concourse.bacc as bacc
import concourse.tile as tile
import concourse.mybir as mybir
from concourse.bass_utils import run_bass_kernel_spmd

N_CORES = 8
R = 256          # rows per core (2048 / 8)
BLK = 32         # rows per DMA block (32 * 64KB = 2 MiB per transfer)
GRP = 4          # rows per PSUM group (4 * 128 f32 = one 2KB PSUM bank)
NBLK = R // BLK
NGRP = BLK // GRP

_cache = {}
LAST_RESULTS = None


def _hadamard128() -> np.ndarray:
    idx = np.arange(128, dtype=np.uint32)
    bits = idx[:, None] & idx[None, :]
    pop = np.zeros_like(bits)
    for s in range(7):
        pop += (bits >> s) & 1
    return np.where(pop % 2 == 0, np.float32(1.0), np.float32(-1.0)).astype(np.float32)


def _build(repeat: int = 1, bench: bool = False, no_compute: bool = False,
           no_dma: bool = False, contig_dma: bool = False, scheme: str = "fp32"):
    nc = bacc.Bacc(
        "TRN2",
        target_bir_lowering=False,
        debug=False,
        num_devices=N_CORES,
    )
    f32 = mybir.dt.float32
    f32r = mybir.dt.float32r
    xdt = f32r if scheme == "f32r" else f32
    x_d = nc.dram_tensor("x", [R, 128, 128], xdt, kind="ExternalInput").ap()
    # h holds [H | H/128 | H] so a 256-wide moving operand starting at col 0
    # gives H-first and one starting at col 128 gives (H/128)-first.
    h_d = nc.dram_tensor("h", [128, 384], xdt, kind="ExternalInput").ap()
    if bench:
        # Timing-only variant: identical DMA traffic, but the real result goes
        # to internal DRAM scratch so the PJRT call only moves a tiny output.
        y_small = nc.dram_tensor("y", [1, 1], f32, kind="ExternalOutput").ap()
    else:
        y_d = nc.dram_tensor("y", [R, 128, 128], f32, kind="ExternalOutput").ap()

    from contextlib import ExitStack

    with tile.TileContext(nc) as tc, ExitStack() as ctx:
        hpool = ctx.enter_context(tc.tile_pool(name="hconst", bufs=1))
        xpool = ctx.enter_context(tc.tile_pool(name="xin", bufs=4))
        ypool = ctx.enter_context(tc.tile_pool(name="yout", bufs=3))
        mpool = ctx.enter_context(tc.tile_pool(name="mid", bufs=4))
        ps1pool = ctx.enter_context(
            tc.tile_pool(name="ps1", bufs=3, space=bass.MemorySpace.PSUM)
        )
        ps2pool = ctx.enter_context(
            tc.tile_pool(name="ps2", bufs=3, space=bass.MemorySpace.PSUM)
        )
        if True:
            if bench:
                dpool = ctx.enter_context(
                    tc.tile_pool(name="dscratch", bufs=1, space=bass.MemorySpace.DRAM)
                )
                y_d = dpool.tile([R, 128, 128], f32)
            ht = hpool.tile([128, 384], xdt)
            nc.sync.dma_start(ht[:], h_d[:])

            for b in range(NBLK * repeat):
                b = b % NBLK
                xt = xpool.tile([128, BLK, 128], xdt)
                if no_dma:
                    # keep the tile "written" so Tile's release pass is happy
                    nc.vector.tensor_copy(xt[:, 0, 0:1], ht[:, 0:1])
                if not no_dma:
                    if contig_dma:
                        nc.sync.dma_start(
                            xt[:],
                            x_d[b * BLK : (b + 1) * BLK].rearrange(
                                "r i j -> (r i j)"
                            ).rearrange("(p n) -> p n", p=128),
                        )
                    else:
                        nc.sync.dma_start(
                            xt[:],
                            x_d[b * BLK : (b + 1) * BLK].rearrange("r i j -> i r j"),
                        )
                yt = ypool.tile([128, BLK, 128], f32)
                if no_compute:
                    nc.vector.tensor_copy(yt[:, 0, 0:1], ht[:, 0:1])
                elif scheme == "fp32":
                    for g in range(NGRP):
                        ps1 = ps1pool.tile([128, GRP, 128], f32)
                        for k in range(GRP):
                            nc.tensor.matmul(
                                ps1[:, k, :],
                                xt[:, g * GRP + k, :],
                                ht[:, 0:128],
                                start=True,
                                stop=True,
                            )
                        sb1 = mpool.tile([128, GRP, 128], f32)
                        nc.vector.tensor_copy(sb1[:], ps1[:])
                        ps2 = ps2pool.tile([128, GRP, 128], f32)
                        for k in range(GRP):
                            nc.tensor.matmul(
                                ps2[:, k, :],
                                sb1[:, k, :],
                                ht[:, 128:256],
                                start=True,
                                stop=True,
                            )
                        nc.scalar.copy(
                            yt[:, g * GRP : (g + 1) * GRP, :], ps2[:]
                        )
                elif scheme == "f32r":
                    G2 = 2  # rows per PSUM group; [128, 2, 256] = one bank
                    rhs1 = ht[:, 0:256]
                    rhs2 = ht[:, 128:384]
                    for g in range(BLK // G2):
                        ps1 = ps1pool.tile([128, G2, 256], f32)
                        for k in range(G2):
                            nc.tensor.matmul(
                                ps1[:, k, :],
                                xt[:, g * G2 + k, :],
                                rhs1,
                                start=True,
                                stop=True,
                            )
                        sb1 = mpool.tile([128, G2, 128], f32r)
                        nc.vector.tensor_copy(sb1[:], ps1[:, :, 0:128])
                        ps2 = ps2pool.tile([128, G2, 256], f32)
                        for k in range(G2):
                            nc.tensor.matmul(
                                ps2[:, k, :],
                                sb1[:, k, :],
                                rhs2,
                                start=True,
                                stop=True,
                            )
                        nc.scalar.copy(
                            yt[:, g * G2 : (g + 1) * G2, :], ps2[:, :, 0:128]
                        )
                else:
                    raise ValueError(scheme)
                if not no_dma:
                    if contig_dma:
                        nc.scalar.dma_start(
                            y_d[b * BLK : (b + 1) * BLK].rearrange(
                                "r a b -> (r a b)"
                            ).rearrange("(p n) -> p n", p=128),
                            yt[:],
                        )
                    else:
                        nc.scalar.dma_start(
                            y_d[b * BLK : (b + 1) * BLK].rearrange("r a b -> a r b"),
                            yt[:],
                        )
            if bench:
                nc.sync.dma_start(y_small[:], ht[:1, :1])

    nc.compile()
    return nc


import os as _os
SCHEME = _os.environ.get("FWHT_SCHEME", "fp32")


def _h_input() -> np.ndarray:
    H = _hadamard128()
    return np.concatenate([H, H / np.float32(128.0), H], axis=1).astype(np.float32)


def kernel(**inputs) -> np.ndarray:
    global LAST_RESULTS
    x = np.ascontiguousarray(np.asarray(inputs["x"], dtype=np.float32))
    B, C, N = x.shape
    assert (B, C, N) == (32, 64, 16384)

    if "nc" not in _cache:
        _cache["nc"] = _build(scheme=SCHEME)
    nc = _cache["nc"]

    H3 = _h_input()
    shards = x.reshape(N_CORES, R, 128, 128)
    in_maps = [
        {"x": np.ascontiguousarray(shards[c]), "h": H3}
        for c in range(N_CORES)
    ]
    res = run_bass_kernel_spmd(nc, in_maps, core_ids=list(range(N_CORES)))
    LAST_RESULTS = res
    y = np.concatenate([res.results[c]["y"].reshape(1, R, 16384) for c in range(N_CORES)])
    return y.reshape(B, C, N)


# revision 23
# speedup vs baseline: 57590.3773x; 51997.1857x over previous
"""FWHT (N=16384, orthonormal) over a (32, 64, 16384) f32 batch on 8 TRN2 cores.

Decomposition: H_16384 = H_128 (x) H_128.  Each length-16384 row reshaped to
X[i, j] (128x128) transforms as Y = H X H / 128.  On the PE (out = lhsT.T @ rhs):
  mm1: lhsT = X  (K=i), rhs = H      -> out1[j, a] = sum_i X[i,j] H[i,a]
  mm2: lhsT = out1 (K=j), rhs = H/128 -> out2[a, b] = Y[a, b]
No transposes needed anywhere; out2 lands in the natural row-major layout.

Sharding: pure data-parallel over the 2048 leading rows -> 256 rows/core.
"""

import numpy as np

import concourse.bass as bass
import concourse.bacc as bacc
import concourse.tile as tile
import concourse.mybir as mybir
from concourse.bass_utils import run_bass_kernel_spmd

N_CORES = 8
R = 256          # rows per core (2048 / 8)
BLK = 32         # rows per DMA block (32 * 64KB = 2 MiB per transfer)
GRP = 4          # rows per PSUM group (4 * 128 f32 = one 2KB PSUM bank)
NBLK = R // BLK
NGRP = BLK // GRP

_cache = {}
LAST_RESULTS = None


def _hadamard128() -> np.ndarray:
    idx = np.arange(128, dtype=np.uint32)
    bits = idx[:, None] & idx[None, :]
    pop = np.zeros_like(bits)
    for s in range(7):
        pop += (bits >> s) & 1
    return np.where(pop % 2 == 0, np.float32(1.0), np.float32(-1.0)).astype(np.float32)


def _build(repeat: int = 1, bench: bool = False, no_compute: bool = False,
           no_dma: bool = False, contig_dma: bool = False, scheme: str = "fp32"):
    nc = bacc.Bacc(
        "TRN2",
        target_bir_lowering=False,
        debug=False,
        num_devices=N_CORES,
    )
    f32 = mybir.dt.float32
    f32r = mybir.dt.float32r
    xdt = f32r if scheme == "f32r" else f32
    x_d = nc.dram_tensor("x", [R, 128, 128], xdt, kind="ExternalInput").ap()
    # h holds [H | H/128 | H] so a 256-wide moving operand starting at col 0
    # gives H-first and one starting at col 128 gives (H/128)-first.
    h_d = nc.dram_tensor("h", [128, 384], xdt, kind="ExternalInput").ap()
    if bench:
        # Timing-only variant: identical DMA traffic, but the real result goes
        # to internal DRAM scratch so the PJRT call only moves a tiny output.
        y_small = nc.dram_tensor("y", [1, 1], f32, kind="ExternalOutput").ap()
    else:
        y_d = nc.dram_tensor("y", [R, 128, 128], f32, kind="ExternalOutput").ap()

    from contextlib import ExitStack

    with tile.TileContext(nc) as tc, ExitStack() as ctx:
        hpool = ctx.enter_context(tc.tile_pool(name="hconst", bufs=1))
        xpool = ctx.enter_context(tc.tile_pool(name="xin", bufs=4))
        ypool = ctx.enter_context(tc.tile_pool(name="yout", bufs=3))
        mpool = ctx.enter_context(tc.tile_pool(name="mid", bufs=6))
        ps1pool = ctx.enter_context(
            tc.tile_pool(name="ps1", bufs=4, space=bass.MemorySpace.PSUM)
        )
        ps2pool = ctx.enter_context(
            tc.tile_pool(name="ps2", bufs=4, space=bass.MemorySpace.PSUM)
        )
        if True:
            if bench:
                dpool = ctx.enter_context(
                    tc.tile_pool(name="dscratch", bufs=1, space=bass.MemorySpace.DRAM)
                )
                y_d = dpool.tile([R, 128, 128], f32)
            ht = hpool.tile([128, 384], xdt)
            nc.sync.dma_start(ht[:], h_d[:])

            for b in range(NBLK * repeat):
                b = b % NBLK
                xt = xpool.tile([128, BLK, 128], xdt)
                if no_dma:
                    # keep the tile "written" so Tile's release pass is happy
                    nc.vector.tensor_copy(xt[:, 0, 0:1], ht[:, 0:1])
                if not no_dma:
                    if contig_dma:
                        nc.sync.dma_start(
                            xt[:],
                            x_d[b * BLK : (b + 1) * BLK].rearrange(
                                "r i j -> (r i j)"
                            ).rearrange("(p n) -> p n", p=128),
                        )
                    else:
                        hb = BLK // 2
                        for h2 in range(2):
                            nc.sync.dma_start(
                                xt[:, h2 * hb : (h2 + 1) * hb, :],
                                x_d[b * BLK + h2 * hb : b * BLK + (h2 + 1) * hb]
                                .rearrange("r i j -> i r j"),
                            )
                yt = ypool.tile([128, BLK, 128], f32)
                if no_compute:
                    nc.vector.tensor_copy(yt[:, 0, 0:1], ht[:, 0:1])
                elif scheme == "fp32":
                    for g in range(NGRP):
                        ps1 = ps1pool.tile([128, GRP, 128], f32)
                        for k in range(GRP):
                            nc.tensor.matmul(
                                ps1[:, k, :],
                                xt[:, g * GRP + k, :],
                                ht[:, 0:128],
                                start=True,
                                stop=True,
                            )
                        sb1 = mpool.tile([128, GRP, 128], f32)
                        nc.vector.tensor_copy(sb1[:], ps1[:])
                        ps2 = ps2pool.tile([128, GRP, 128], f32)
                        for k in range(GRP):
                            nc.tensor.matmul(
                                ps2[:, k, :],
                                sb1[:, k, :],
                                ht[:, 128:256],
                                start=True,
                                stop=True,
                            )
                        nc.scalar.copy(
                            yt[:, g * GRP : (g + 1) * GRP, :], ps2[:]
                        )
                elif scheme == "f32r":
                    G2 = 2  # rows per PSUM group; [128, 2, 256] = one bank
                    rhs1 = ht[:, 0:256]
                    rhs2 = ht[:, 128:384]
                    for g in range(BLK // G2):
                        ps1 = ps1pool.tile([128, G2, 256], f32)
                        for k in range(G2):
                            nc.tensor.matmul(
                                ps1[:, k, :],
                                xt[:, g * G2 + k, :],
                                rhs1,
                                start=True,
                                stop=True,
                            )
                        sb1 = mpool.tile([128, G2, 128], f32r)
                        nc.vector.tensor_copy(sb1[:], ps1[:, :, 0:128])
                        ps2 = ps2pool.tile([128, G2, 256], f32)
                        for k in range(G2):
                            nc.tensor.matmul(
                                ps2[:, k, :],
                                sb1[:, k, :],
                                rhs2,
                                start=True,
                                stop=True,
                            )
                        nc.scalar.copy(
                            yt[:, g * G2 : (g + 1) * G2, :], ps2[:, :, 0:128]
                        )
                else:
                    raise ValueError(scheme)
                if not no_dma:
                    if contig_dma:
                        nc.scalar.dma_start(
                            y_d[b * BLK : (b + 1) * BLK].rearrange(
                                "r a b -> (r a b)"
                            ).rearrange("(p n) -> p n", p=128),
                            yt[:],
                        )
                    else:
                        hb = BLK // 2
                        for h2 in range(2):
                            nc.scalar.dma_start(
                                y_d[b * BLK + h2 * hb : b * BLK + (h2 + 1) * hb]
                                .rearrange("r a b -> a r b"),
                                yt[:, h2 * hb : (h2 + 1) * hb, :],
                            )
            if bench:
                nc.sync.dma_start(y_small[:], ht[:1, :1])

    nc.compile()
    return nc


import os as _os
# fp32 is the production scheme (rel err ~2e-7 vs reference); "f32r" runs the
# PE 2x faster but is TF32-class accurate (~1.5e-4) — not worth the risk.
SCHEME = _os.environ.get("FWHT_SCHEME", "fp32")


def _h_input() -> np.ndarray:
    H = _hadamard128()
    return np.concatenate([H, H / np.float32(128.0), H], axis=1).astype(np.float32)


def kernel(**inputs) -> np.ndarray:
    global LAST_RESULTS
    # NTFF tracing is unavailable under this axon tunnel (antenv.axon_hooks
    # missing) and would crash run_bass_kernel_spmd if BASS_TRACE leaked in.
    _os.environ["BASS_NEVER_TRACE"] = "1"
    x = np.ascontiguousarray(np.asarray(inputs["x"], dtype=np.float32))
    B, C, N = x.shape
    assert (B, C, N) == (32, 64, 16384)

    if "nc" not in _cache:
        _cache["nc"] = _build(scheme=SCHEME)
    nc = _cache["nc"]

    H3 = _h_input()
    shards = x.reshape(N_CORES, R, 128, 128)
    in_maps = [
        {"x": np.ascontiguousarray(shards[c]), "h": H3}
        for c in range(N_CORES)
    ]
    res = run_bass_kernel_spmd(nc, in_maps, core_ids=list(range(N_CORES)))
    LAST_RESULTS = res
    y = np.concatenate([res.results[c]["y"].reshape(1, R, 16384) for c in range(N_CORES)])
    return y.reshape(B, C, N)
